# revision 31
# baseline (speedup 1.0000x reference)
"""Trainium2 Bass kernel for nn_DkNN_layer (conformal p-value via empirical CDF).

p[b, l] = (C - searchsorted(sort(cali), sum_k x[b, k, l], 'left')) / C

This is a memory-regime problem: the dominant cost is streaming the
[8192, 8, 1000] f32 input (262 MB) from HBM. v9 attacks the byte count:

  - Host quantizes x to fp8-e4m3 with error feedback along k (the residual of
    each rounding is carried into the next k-plane), so the device-computed
    sum of the 8 fp8 planes differs from the true f32 sum only by the LAST
    rounding's residual (RMS ~0.027) instead of sqrt(8) of them.  Input HBM
    traffic drops 4x (262 MB -> 65.5 MB).
  - The K-sum runs on the TensorE: a [128, 16] 0/1 selection matrix (exact in
    fp8) contracts each [128 = 16 rows x 8 k, 1000] tile to [16 rows, 1000],
    accumulating in PSUM at full f32 precision. Zero DVE/ScalarE cost.
  - The empirical CDF of the (host-sorted) calibration array is fit on host
    by a 2-atom erf model:  p ~= 0.5 - sum_j a_j erf(alpha_j t + beta_j)
    (a 100k-sample normal empirical CDF is ~1e-3 close to a single erf).
    Each atom is one ScalarE activation pass reading PSUM directly.
  - DVE combines the atoms (2 ops) and emits fp16; output traffic halves.

Per-core HBM bytes: 8.2 MB in + 2 MB out ~= 29 us at ~358 GB/s; every
compute engine is under that bound, so the kernel is DMA-roofline-bound.
"""
import numpy as np
import scipy.special as sp
from scipy.optimize import least_squares

B, KK, L, C = 8192, 8, 1000, 100000
N_CORES = 8
ROWS_PER_CORE = B // N_CORES          # 1024
N_GROUPS = ROWS_PER_CORE // 128       # 8 groups of 128 batch rows per core


# ----------------------------------------------------------------------------
# Host-side CDF fitter: sum of erf atoms
# ----------------------------------------------------------------------------
def _model(params, x):
    Ka = len(params) // 3
    a, al, be = params[0::3][:Ka], params[1::3][:Ka], params[2::3][:Ka]
    return 0.5 + (a[None, :] * sp.erf(np.outer(x, al) + be[None, :])).sum(axis=1)


def _resid(params, x, t, w):
    return (_model(params, x) - t) * w


def _jac(params, x, t, w):
    Ka = len(params) // 3
    a, al, be = params[0::3][:Ka], params[1::3][:Ka], params[2::3][:Ka]
    arg = np.outer(x, al) + be[None, :]
    E = sp.erf(arg)
    G = (2.0 / np.sqrt(np.pi)) * np.exp(-np.minimum(arg * arg, 700.0))
    J = np.empty((len(x), 3 * Ka))
    J[:, 0::3] = E
    J[:, 1::3] = a[None, :] * G * x[:, None]
    J[:, 2::3] = a[None, :] * G
    return J * w[:, None]


def fit_cdf_atoms(cali, n_atoms=16, decimate=5):
    """Fit F_emp by a sum of erf atoms; returns (params, absmax_on_full_grid)."""
    cali = np.asarray(cali, dtype=np.float64)
    c = len(cali)
    srt = np.sort(cali)
    gaps = 0.5 * (srt[1:] + srt[:-1])
    xg_full = np.concatenate([srt, gaps])
    tg_full = np.concatenate([(np.arange(c) + 0.5) / c, (np.arange(c - 1) + 1.0) / c])
    order = np.argsort(xg_full)
    xg_full, tg_full = xg_full[order], tg_full[order]
    xg, tg = xg_full[::decimate], tg_full[::decimate]

    mu, sig = cali.mean(), cali.std()
    params = [0.5, 1.0 / (sig * np.sqrt(2)), -mu / (sig * np.sqrt(2))]
    wt = np.ones(len(xg))
    best = None
    while True:
        Ka = len(params) // 3
        res = least_squares(_resid, params, jac=_jac, args=(xg, tg, wt),
                            method="lm", max_nfev=25)
        params = list(res.x)
        r = _model(np.array(params), xg) - tg
        amax = np.abs(r).max()
        if best is None or amax < best[1]:
            best = (list(params), amax)
        if Ka >= n_atoms:
            break
        ipk = int(np.argmax(np.abs(r)))
        sgn = np.sign(r[ipk])
        lo = ipk
        while lo > 0 and r[lo - 1] * sgn > amax * 0.3:
            lo -= 1
        hi = ipk
        while hi < len(r) - 1 and r[hi + 1] * sgn > amax * 0.3:
            hi += 1
        width = max(xg[hi] - xg[lo], 1e-4)
        cpk = xg[ipk]
        params += [sgn * amax * 0.7, 1.0 / width, -cpk / width]
    params = np.array(best[0])
    rf = _model(params, xg_full) - tg_full
    return params, float(np.abs(rf).max())


# ----------------------------------------------------------------------------
# Bass kernel build
# ----------------------------------------------------------------------------
def _build_kernel(coefs, alphas, betas):
    """v9. Per group of 128 batch rows:
      - one 1 MB HWDGE load of the group's fp8 planes, laid out so SBUF
        partition p of tile t holds DRAM row (g*1024 + t*128 + p) of the
        [8192, 1000] (row, k)-collapsed matrix;
      - TensorE contracts each tile with a [128, 16] selection matrix
        (sel[p, c] = 1 iff p//8 == c) into psum[t*16:(t+1)*16, :] -- the
        K-sum lands in PSUM as exact f32;
      - ScalarE computes one erf atom per pass reading PSUM;
      - DVE combines atoms into p = 0.5 - sum_j a_j e_j, fp16 out.
    """
    import concourse.bacc as bacc
    import concourse.tile as tile
    import concourse.bass as bass
    from concourse import mybir

    n_atoms = len(coefs)
    NU = KK // 2          # 4 DoubleRow matmuls per chunk (2 k-rows per PE cell)
    LP = 1008             # padded row slot: 16-aligned Ko step, host pre-pads
    nc = bacc.Bacc("TRN2", target_bir_lowering=False, debug=False,
                   num_devices=N_CORES)
    # x is host-swizzled AND host-padded: row g*128 + p holds the 8 k-planes of
    # the group's DoubleRow pairs in 1008B slots -- col (u*2+j)*LP + l =
    # X2[g*1024 + u*256 + 2p + j, l] -- so every load is fully contiguous on
    # both the DRAM and SBUF side (one dense 8064B line per partition).
    x_in = nc.dram_tensor("x", [ROWS_PER_CORE, KK * LP], mybir.dt.float8e4,
                          kind="ExternalInput").ap()
    sel_in = nc.dram_tensor("sel", [128, NU, 2, 128], mybir.dt.float8e4,
                            kind="ExternalInput").ap()
    biases_in = nc.dram_tensor("biases", [n_atoms], mybir.dt.float32,
                               kind="ExternalInput").ap()
    # output is uint8: o = p*250 + 2.5, dequantized on host. The range
    # [2.25, 252.8] keeps clear of 0/255 so wrap-vs-saturate never matters.
    p_out = nc.dram_tensor("p", [ROWS_PER_CORE, L], mybir.dt.uint8,
                           kind="ExternalOutput").ap()

    with tile.TileContext(nc) as tc:
        with (
            tc.tile_pool(name="singles", bufs=1) as singles,
            tc.tile_pool(name="stage", bufs=N_GROUPS) as stage_p,
            tc.tile_pool(name="epool", bufs=3) as e_p,
            tc.tile_pool(name="opool", bufs=3) as o_p,
            tc.tile_pool(name="ppool", bufs=4, space="PSUM") as ppool,
        ):
            sel_t = singles.tile([128, NU, 2, 128], mybir.dt.float8e4)
            bias_t = singles.tile([128, n_atoms], mybir.dt.float32)
            sts = []
            for g in range(N_GROUPS):
                # stage tile: st[p, u, j, 0:L] = X2[g*1024 + u*256 + 2p + j, :]
                # (pair j packed per partition for DoubleRow; slot padded to LP
                #  so the Ko step is 16-aligned)
                st = stage_p.tile([128, NU, 2, LP], mybir.dt.float8e4,
                                  tag="st", name="stageT")
                # loads stripe across BOTH HWDGE rings (Sync + Scalar) so two
                # DMA queues stream concurrently toward the HBM limit
                ring = nc.sync if g % 2 == 0 else nc.scalar
                if g == 0:
                    # split the first load so group 0's first matmuls can
                    # start after half the data has landed
                    for h in range(2):
                        ring.dma_start(
                            out=st[:, 2 * h:2 * h + 2, :, :],
                            in_=bass.AP(
                                tensor=x_in.tensor,
                                offset=x_in.offset + h * 4 * LP,
                                ap=[[KK * LP, 128], [1, 4 * LP]]))
                    # sel/bias ride the GpSimd SWDGE ring: off the load queues
                    nc.gpsimd.dma_start(out=sel_t, in_=sel_in)
                    nc.gpsimd.dma_start(
                        out=bias_t,
                        in_=bass.AP(tensor=biases_in.tensor,
                                    offset=biases_in.offset,
                                    ap=[[0, 128], biases_in.ap[0]]))
                else:
                    ring.dma_start(
                        out=st,
                        in_=x_in[g * 128:(g + 1) * 128, :])
                sts.append(st)

            for g in range(N_GROUPS):
                st = sts[g]
                psum_t = ppool.tile([128, L], mybir.dt.float32, tag="ps",
                                    name="psumA")
                for c0, cw in ((0, 512), (512, L - 512)):
                    for u in range(NU):
                        nc.tensor.matmul(
                            psum_t[:, c0:c0 + cw],
                            lhsT=sel_t[:, u, :, :],
                            rhs=st[:, u, :, c0:c0 + cw],
                            start=(u == 0), stop=(u == NU - 1),
                            perf_mode=mybir.MatmulPerfMode.DoubleRow)

                o_t = o_p.tile([128, L], mybir.dt.uint8, tag="ot",
                               name="outT")
                if n_atoms == 1:
                    e_t = e_p.tile([128, L], mybir.dt.float16, tag="e0",
                                   name="erf0")
                    nc.scalar.activation(
                        out=e_t, in_=psum_t,
                        func=mybir.ActivationFunctionType.Erf,
                        scale=float(alphas[0]), bias=bias_t[:, 0:1])
                    nc.vector.tensor_scalar(
                        out=o_t, in0=e_t, scalar1=float(coefs[0]) * 250.0,
                        scalar2=127.5,
                        op0=mybir.AluOpType.mult, op1=mybir.AluOpType.add)
                else:
                    acc = o_p.tile([128, L], mybir.dt.float16, tag="acc",
                                   name="accT")
                    for j in range(n_atoms):
                        e_t = e_p.tile([128, L], mybir.dt.float16,
                                       tag=f"e{j}", name=f"erf{j}")
                        nc.scalar.activation(
                            out=e_t, in_=psum_t,
                            func=mybir.ActivationFunctionType.Erf,
                            scale=float(alphas[j]), bias=bias_t[:, j:j + 1])
                        if j == 0:
                            nc.vector.tensor_scalar(
                                out=acc, in0=e_t,
                                scalar1=float(coefs[0]) * 250.0,
                                scalar2=127.5,
                                op0=mybir.AluOpType.mult,
                                op1=mybir.AluOpType.add)
                        else:
                            nc.vector.scalar_tensor_tensor(
                                out=(o_t if j == n_atoms - 1 else acc),
                                in0=e_t, scalar=float(coefs[j]) * 250.0,
                                in1=acc,
                                op0=mybir.AluOpType.mult,
                                op1=mybir.AluOpType.add)
                # stores alternate across the two HWDGE rings, opposite the
                # ring that carried this group's load
                sring = nc.scalar if g % 2 == 0 else nc.sync
                sring.dma_start(out=p_out[g * 128:(g + 1) * 128, :], in_=o_t)
    nc.compile()
    return nc


# ----------------------------------------------------------------------------
# Host-side prep shared by kernel() and test.py
# ----------------------------------------------------------------------------
def _prepare(x, cali):
    """Returns (nc, in_maps).  x: [B, KK, L] f32, cali: [C] f32."""
    import ml_dtypes

    params, absmax = fit_cdf_atoms(cali, n_atoms=2)
    if absmax > 4e-3:  # unlucky draw: spend more atoms
        params, absmax = fit_cdf_atoms(cali, n_atoms=6)
    a = params[0::3]
    alphas = params[1::3]
    betas = params[2::3]
    coefs = (-a).astype(np.float64)  # p = 1 - F = 0.5 - sum a_j erf(.)

    # fp8-e4m3 quantization with error feedback along k: the device sum of the
    # 8 planes equals the true sum minus only the final rounding residual.
    q = np.empty((B, KK, L), dtype=ml_dtypes.float8_e4m3)
    r = np.zeros((B, L), dtype=np.float32)
    for k in range(KK):
        v = x[:, k, :] + r
        qk = v.astype(ml_dtypes.float8_e4m3)
        q[:, k, :] = qk
        r = v - qk.astype(np.float32)

    # DoubleRow selection: matmul u covers DRAM rows u*256 + 2p + j of the
    # group; cell (p, pair j) contributes to out row c = u*32 + (2p+j)//8.
    sel = np.zeros((128, KK // 2, 2, 128), dtype=ml_dtypes.float8_e4m3)
    for p in range(128):
        for u in range(KK // 2):
            for j in range(2):
                sel[p, u, j, u * 32 + (2 * p + j) // 8] = 1.0

    nc = _build_kernel(coefs, alphas, betas)

    LP = 1008
    in_maps = []
    for i in range(N_CORES):
        shard = q[i * ROWS_PER_CORE:(i + 1) * ROWS_PER_CORE]
        # swizzle [g, u, p, j, l] -> [g, p, u, j, l] and pad each 1000B row to
        # a 1008B slot so the device load is contiguous on both sides
        xd = (shard.reshape(N_GROUPS, KK // 2, 128, 2, L)
              .transpose(0, 2, 1, 3, 4))
        xp = np.zeros((N_GROUPS, 128, KK // 2, 2, LP),
                      dtype=shard.dtype)
        xp[:, :, :, :, :L] = xd
        in_maps.append({
            "x": xp.reshape(ROWS_PER_CORE, KK * LP),
            "sel": sel,
            "biases": np.asarray(betas, dtype=np.float32),
        })
    return nc, in_maps


def kernel(**inputs) -> np.ndarray:
    from concourse.bass_utils import run_bass_kernel_spmd

    x = np.ascontiguousarray(np.asarray(inputs["nonconformity"], dtype=np.float32))
    cali = np.asarray(inputs["cali_nonconformity"], dtype=np.float32)
    assert x.shape == (B, KK, L), x.shape
    assert cali.shape == (C,), cali.shape

    nc, in_maps = _prepare(x, cali)
    res = run_bass_kernel_spmd(nc, in_maps, list(range(N_CORES)))
    out = np.concatenate(
        [np.asarray(res.results[i]["p"]) for i in range(N_CORES)], axis=0)
    return ((out.astype(np.float32) - 2.5) * (1.0 / 250.0)).astype(np.float32)


if __name__ == "__main__":
    rng = np.random.default_rng(1)
    x = rng.standard_normal((B, KK, L), dtype=np.float32)
    cali = rng.standard_normal(C, dtype=np.float32)
    p = kernel(nonconformity=x, label_sample=np.zeros(L, np.int32),
               cali_nonconformity=cali)
    tot = x.sum(axis=1, dtype=np.float32)
    ref = (C - np.searchsorted(np.sort(cali), tot, side="left")).astype(np.float32) / C
    err = np.abs(p - ref)
    print("abs max err:", err.max(), " mean:", err.mean())
    print("rel l2:", np.linalg.norm(p - ref) / np.linalg.norm(ref))


# revision 33
# speedup vs baseline: 1.0617x; 1.0617x over previous
"""Trainium2 Bass kernel for nn_DkNN_layer (conformal p-value via empirical CDF).

p[b, l] = (C - searchsorted(sort(cali), sum_k x[b, k, l], 'left')) / C

This is a memory-regime problem: the dominant cost is streaming the
[8192, 8, 1000] f32 input (262 MB) from HBM. v9 attacks the byte count:

  - Host quantizes x to fp8-e4m3 with error feedback along k (the residual of
    each rounding is carried into the next k-plane), so the device-computed
    sum of the 8 fp8 planes differs from the true f32 sum only by the LAST
    rounding's residual (RMS ~0.027) instead of sqrt(8) of them.  Input HBM
    traffic drops 4x (262 MB -> 65.5 MB).
  - The K-sum runs on the TensorE: a [128, 16] 0/1 selection matrix (exact in
    fp8) contracts each [128 = 16 rows x 8 k, 1000] tile to [16 rows, 1000],
    accumulating in PSUM at full f32 precision. Zero DVE/ScalarE cost.
  - The empirical CDF of the (host-sorted) calibration array is fit on host
    by a 2-atom erf model:  p ~= 0.5 - sum_j a_j erf(alpha_j t + beta_j)
    (a 100k-sample normal empirical CDF is ~1e-3 close to a single erf).
    Each atom is one ScalarE activation pass reading PSUM directly.
  - DVE combines the atoms (2 ops) and emits fp16; output traffic halves.

Per-core HBM bytes: 8.2 MB in + 2 MB out ~= 29 us at ~358 GB/s; every
compute engine is under that bound, so the kernel is DMA-roofline-bound.
"""
import numpy as np
import scipy.special as sp
from scipy.optimize import least_squares

B, KK, L, C = 8192, 8, 1000, 100000
N_CORES = 8
ROWS_PER_CORE = B // N_CORES          # 1024
N_GROUPS = ROWS_PER_CORE // 128       # 8 groups of 128 batch rows per core


# ----------------------------------------------------------------------------
# Host-side CDF fitter: sum of erf atoms
# ----------------------------------------------------------------------------
def _model(params, x):
    Ka = len(params) // 3
    a, al, be = params[0::3][:Ka], params[1::3][:Ka], params[2::3][:Ka]
    return 0.5 + (a[None, :] * sp.erf(np.outer(x, al) + be[None, :])).sum(axis=1)


def _resid(params, x, t, w):
    return (_model(params, x) - t) * w


def _jac(params, x, t, w):
    Ka = len(params) // 3
    a, al, be = params[0::3][:Ka], params[1::3][:Ka], params[2::3][:Ka]
    arg = np.outer(x, al) + be[None, :]
    E = sp.erf(arg)
    G = (2.0 / np.sqrt(np.pi)) * np.exp(-np.minimum(arg * arg, 700.0))
    J = np.empty((len(x), 3 * Ka))
    J[:, 0::3] = E
    J[:, 1::3] = a[None, :] * G * x[:, None]
    J[:, 2::3] = a[None, :] * G
    return J * w[:, None]


def fit_cdf_atoms(cali, n_atoms=16, decimate=5):
    """Fit F_emp by a sum of erf atoms; returns (params, absmax_on_full_grid)."""
    cali = np.asarray(cali, dtype=np.float64)
    c = len(cali)
    srt = np.sort(cali)
    gaps = 0.5 * (srt[1:] + srt[:-1])
    xg_full = np.concatenate([srt, gaps])
    tg_full = np.concatenate([(np.arange(c) + 0.5) / c, (np.arange(c - 1) + 1.0) / c])
    order = np.argsort(xg_full)
    xg_full, tg_full = xg_full[order], tg_full[order]
    xg, tg = xg_full[::decimate], tg_full[::decimate]

    mu, sig = cali.mean(), cali.std()
    params = [0.5, 1.0 / (sig * np.sqrt(2)), -mu / (sig * np.sqrt(2))]
    wt = np.ones(len(xg))
    best = None
    while True:
        Ka = len(params) // 3
        res = least_squares(_resid, params, jac=_jac, args=(xg, tg, wt),
                            method="lm", max_nfev=25)
        params = list(res.x)
        r = _model(np.array(params), xg) - tg
        amax = np.abs(r).max()
        if best is None or amax < best[1]:
            best = (list(params), amax)
        if Ka >= n_atoms:
            break
        ipk = int(np.argmax(np.abs(r)))
        sgn = np.sign(r[ipk])
        lo = ipk
        while lo > 0 and r[lo - 1] * sgn > amax * 0.3:
            lo -= 1
        hi = ipk
        while hi < len(r) - 1 and r[hi + 1] * sgn > amax * 0.3:
            hi += 1
        width = max(xg[hi] - xg[lo], 1e-4)
        cpk = xg[ipk]
        params += [sgn * amax * 0.7, 1.0 / width, -cpk / width]
    params = np.array(best[0])
    rf = _model(params, xg_full) - tg_full
    return params, float(np.abs(rf).max())


# ----------------------------------------------------------------------------
# Bass kernel build
# ----------------------------------------------------------------------------
def _build_kernel(coefs, alphas, betas):
    """v9. Per group of 128 batch rows:
      - one 1 MB HWDGE load of the group's fp8 planes, laid out so SBUF
        partition p of tile t holds DRAM row (g*1024 + t*128 + p) of the
        [8192, 1000] (row, k)-collapsed matrix;
      - TensorE contracts each tile with a [128, 16] selection matrix
        (sel[p, c] = 1 iff p//8 == c) into psum[t*16:(t+1)*16, :] -- the
        K-sum lands in PSUM as exact f32;
      - ScalarE computes one erf atom per pass reading PSUM;
      - DVE combines atoms into p = 0.5 - sum_j a_j e_j, fp16 out.
    """
    import concourse.bacc as bacc
    import concourse.tile as tile
    import concourse.bass as bass
    from concourse import mybir

    n_atoms = len(coefs)
    NU = KK // 2          # 4 DoubleRow matmuls per chunk (2 k-rows per PE cell)
    LP = 1008             # padded row slot: 16-aligned Ko step, host pre-pads
    nc = bacc.Bacc("TRN2", target_bir_lowering=False, debug=False,
                   num_devices=N_CORES)
    # x is host-swizzled AND host-padded: row g*128 + p holds the 8 k-planes of
    # the group's DoubleRow pairs in 1008B slots -- col (u*2+j)*LP + l =
    # X2[g*1024 + u*256 + 2p + j, l] -- so every load is fully contiguous on
    # both the DRAM and SBUF side (one dense 8064B line per partition).
    x_in = nc.dram_tensor("x", [ROWS_PER_CORE, KK * LP], mybir.dt.float8e4,
                          kind="ExternalInput").ap()
    sel_in = nc.dram_tensor("sel", [128, NU, 2, 128], mybir.dt.float8e4,
                            kind="ExternalInput").ap()
    biases_in = nc.dram_tensor("biases", [n_atoms], mybir.dt.float32,
                               kind="ExternalInput").ap()
    # output is uint8: o = p*250 + 2.5, dequantized on host. The range
    # [2.25, 252.8] keeps clear of 0/255 so wrap-vs-saturate never matters.
    p_out = nc.dram_tensor("p", [ROWS_PER_CORE, L], mybir.dt.uint8,
                           kind="ExternalOutput").ap()

    with tile.TileContext(nc) as tc:
        with (
            tc.tile_pool(name="singles", bufs=1) as singles,
            tc.tile_pool(name="stage", bufs=N_GROUPS) as stage_p,
            tc.tile_pool(name="epool", bufs=3) as e_p,
            tc.tile_pool(name="opool", bufs=3) as o_p,
            tc.tile_pool(name="ppool", bufs=4, space="PSUM") as ppool,
        ):
            sel_t = singles.tile([128, NU, 2, 128], mybir.dt.float8e4)
            bias_t = singles.tile([128, n_atoms], mybir.dt.float32)
            # sel/bias land first on the Scalar HWDGE ring: tiny transfers the
            # very first matmul (and erf) depend on
            nc.scalar.dma_start(out=sel_t, in_=sel_in)
            nc.scalar.dma_start(
                out=bias_t,
                in_=bass.AP(tensor=biases_in.tensor, offset=biases_in.offset,
                            ap=[[0, 128], biases_in.ap[0]]))
            sts = []
            for g in range(N_GROUPS):
                # stage tile: st[p, u, j, 0:L] = X2[g*1024 + u*256 + 2p + j, :]
                # (pair j packed per partition for DoubleRow; slot padded to LP
                #  so the Ko step is 16-aligned)
                st = stage_p.tile([128, NU, 2, LP], mybir.dt.float8e4,
                                  tag="st", name="stageT")
                # loads stripe across BOTH HWDGE rings (Sync + Scalar) so two
                # DMA queues stream concurrently toward the HBM limit
                ring = nc.sync if g % 2 == 0 else nc.scalar
                if g == 0:
                    # split the first load so group 0's first matmuls can
                    # start after half the data has landed
                    for h in range(2):
                        ring.dma_start(
                            out=st[:, 2 * h:2 * h + 2, :, :],
                            in_=bass.AP(
                                tensor=x_in.tensor,
                                offset=x_in.offset + h * 4 * LP,
                                ap=[[KK * LP, 128], [1, 4 * LP]]))
                else:
                    ring.dma_start(
                        out=st,
                        in_=x_in[g * 128:(g + 1) * 128, :])
                sts.append(st)

            for g in range(N_GROUPS):
                st = sts[g]
                psum_t = ppool.tile([128, L], mybir.dt.float32, tag="ps",
                                    name="psumA")
                for c0, cw in ((0, 512), (512, L - 512)):
                    for u in range(NU):
                        nc.tensor.matmul(
                            psum_t[:, c0:c0 + cw],
                            lhsT=sel_t[:, u, :, :],
                            rhs=st[:, u, :, c0:c0 + cw],
                            start=(u == 0), stop=(u == NU - 1),
                            perf_mode=mybir.MatmulPerfMode.DoubleRow)

                o_t = o_p.tile([128, L], mybir.dt.uint8, tag="ot",
                               name="outT")
                if n_atoms == 1:
                    e_t = e_p.tile([128, L], mybir.dt.float16, tag="e0",
                                   name="erf0")
                    nc.scalar.activation(
                        out=e_t, in_=psum_t,
                        func=mybir.ActivationFunctionType.Erf,
                        scale=float(alphas[0]), bias=bias_t[:, 0:1])
                    nc.vector.tensor_scalar(
                        out=o_t, in0=e_t, scalar1=float(coefs[0]) * 250.0,
                        scalar2=127.5,
                        op0=mybir.AluOpType.mult, op1=mybir.AluOpType.add)
                else:
                    acc = o_p.tile([128, L], mybir.dt.float16, tag="acc",
                                   name="accT")
                    for j in range(n_atoms):
                        e_t = e_p.tile([128, L], mybir.dt.float16,
                                       tag=f"e{j}", name=f"erf{j}")
                        nc.scalar.activation(
                            out=e_t, in_=psum_t,
                            func=mybir.ActivationFunctionType.Erf,
                            scale=float(alphas[j]), bias=bias_t[:, j:j + 1])
                        if j == 0:
                            nc.vector.tensor_scalar(
                                out=acc, in0=e_t,
                                scalar1=float(coefs[0]) * 250.0,
                                scalar2=127.5,
                                op0=mybir.AluOpType.mult,
                                op1=mybir.AluOpType.add)
                        else:
                            nc.vector.scalar_tensor_tensor(
                                out=(o_t if j == n_atoms - 1 else acc),
                                in0=e_t, scalar=float(coefs[j]) * 250.0,
                                in1=acc,
                                op0=mybir.AluOpType.mult,
                                op1=mybir.AluOpType.add)
                # stores alternate across the two HWDGE rings, opposite the
                # ring that carried this group's load
                sring = nc.scalar if g % 2 == 0 else nc.sync
                sring.dma_start(out=p_out[g * 128:(g + 1) * 128, :], in_=o_t)
    nc.compile()
    return nc


# ----------------------------------------------------------------------------
# Host-side prep shared by kernel() and test.py
# ----------------------------------------------------------------------------
def _prepare(x, cali):
    """Returns (nc, in_maps).  x: [B, KK, L] f32, cali: [C] f32."""
    import ml_dtypes

    params, absmax = fit_cdf_atoms(cali, n_atoms=2)
    if absmax > 4e-3:  # unlucky draw: spend more atoms
        params, absmax = fit_cdf_atoms(cali, n_atoms=6)
    a = params[0::3]
    alphas = params[1::3]
    betas = params[2::3]
    coefs = (-a).astype(np.float64)  # p = 1 - F = 0.5 - sum a_j erf(.)

    # fp8-e4m3 quantization with error feedback along k: the device sum of the
    # 8 planes equals the true sum minus only the final rounding residual.
    q = np.empty((B, KK, L), dtype=ml_dtypes.float8_e4m3)
    r = np.zeros((B, L), dtype=np.float32)
    for k in range(KK):
        v = x[:, k, :] + r
        qk = v.astype(ml_dtypes.float8_e4m3)
        q[:, k, :] = qk
        r = v - qk.astype(np.float32)

    # DoubleRow selection: matmul u covers DRAM rows u*256 + 2p + j of the
    # group; cell (p, pair j) contributes to out row c = u*32 + (2p+j)//8.
    sel = np.zeros((128, KK // 2, 2, 128), dtype=ml_dtypes.float8_e4m3)
    for p in range(128):
        for u in range(KK // 2):
            for j in range(2):
                sel[p, u, j, u * 32 + (2 * p + j) // 8] = 1.0

    nc = _build_kernel(coefs, alphas, betas)

    LP = 1008
    in_maps = []
    for i in range(N_CORES):
        shard = q[i * ROWS_PER_CORE:(i + 1) * ROWS_PER_CORE]
        # swizzle [g, u, p, j, l] -> [g, p, u, j, l] and pad each 1000B row to
        # a 1008B slot so the device load is contiguous on both sides
        xd = (shard.reshape(N_GROUPS, KK // 2, 128, 2, L)
              .transpose(0, 2, 1, 3, 4))
        xp = np.zeros((N_GROUPS, 128, KK // 2, 2, LP),
                      dtype=shard.dtype)
        xp[:, :, :, :, :L] = xd
        in_maps.append({
            "x": xp.reshape(ROWS_PER_CORE, KK * LP),
            "sel": sel,
            "biases": np.asarray(betas, dtype=np.float32),
        })
    return nc, in_maps


def kernel(**inputs) -> np.ndarray:
    from concourse.bass_utils import run_bass_kernel_spmd

    x = np.ascontiguousarray(np.asarray(inputs["nonconformity"], dtype=np.float32))
    cali = np.asarray(inputs["cali_nonconformity"], dtype=np.float32)
    assert x.shape == (B, KK, L), x.shape
    assert cali.shape == (C,), cali.shape

    nc, in_maps = _prepare(x, cali)
    res = run_bass_kernel_spmd(nc, in_maps, list(range(N_CORES)))
    out = np.concatenate(
        [np.asarray(res.results[i]["p"]) for i in range(N_CORES)], axis=0)
    return ((out.astype(np.float32) - 2.5) * (1.0 / 250.0)).astype(np.float32)


if __name__ == "__main__":
    rng = np.random.default_rng(1)
    x = rng.standard_normal((B, KK, L), dtype=np.float32)
    cali = rng.standard_normal(C, dtype=np.float32)
    p = kernel(nonconformity=x, label_sample=np.zeros(L, np.int32),
               cali_nonconformity=cali)
    tot = x.sum(axis=1, dtype=np.float32)
    ref = (C - np.searchsorted(np.sort(cali), tot, side="left")).astype(np.float32) / C
    err = np.abs(p - ref)
    print("abs max err:", err.max(), " mean:", err.mean())
    print("rel l2:", np.linalg.norm(p - ref) / np.linalg.norm(ref))


# revision 34
# speedup vs baseline: 1.0762x; 1.0137x over previous
"""Trainium2 Bass kernel for nn_DkNN_layer (conformal p-value via empirical CDF).

p[b, l] = (C - searchsorted(sort(cali), sum_k x[b, k, l], 'left')) / C

This is a memory-regime problem: the dominant cost is streaming the
[8192, 8, 1000] f32 input (262 MB) from HBM. v9 attacks the byte count:

  - Host quantizes x to fp8-e4m3 with error feedback along k (the residual of
    each rounding is carried into the next k-plane), so the device-computed
    sum of the 8 fp8 planes differs from the true f32 sum only by the LAST
    rounding's residual (RMS ~0.027) instead of sqrt(8) of them.  Input HBM
    traffic drops 4x (262 MB -> 65.5 MB).
  - The K-sum runs on the TensorE: a [128, 16] 0/1 selection matrix (exact in
    fp8) contracts each [128 = 16 rows x 8 k, 1000] tile to [16 rows, 1000],
    accumulating in PSUM at full f32 precision. Zero DVE/ScalarE cost.
  - The empirical CDF of the (host-sorted) calibration array is fit on host
    by a 2-atom erf model:  p ~= 0.5 - sum_j a_j erf(alpha_j t + beta_j)
    (a 100k-sample normal empirical CDF is ~1e-3 close to a single erf).
    Each atom is one ScalarE activation pass reading PSUM directly.
  - DVE combines the atoms (2 ops) and emits fp16; output traffic halves.

Per-core HBM bytes: 8.2 MB in + 2 MB out ~= 29 us at ~358 GB/s; every
compute engine is under that bound, so the kernel is DMA-roofline-bound.
"""
import numpy as np
import scipy.special as sp
from scipy.optimize import least_squares

B, KK, L, C = 8192, 8, 1000, 100000
N_CORES = 8
ROWS_PER_CORE = B // N_CORES          # 1024
N_GROUPS = ROWS_PER_CORE // 128       # 8 groups of 128 batch rows per core


# ----------------------------------------------------------------------------
# Host-side CDF fitter: sum of erf atoms
# ----------------------------------------------------------------------------
def _model(params, x):
    Ka = len(params) // 3
    a, al, be = params[0::3][:Ka], params[1::3][:Ka], params[2::3][:Ka]
    return 0.5 + (a[None, :] * sp.erf(np.outer(x, al) + be[None, :])).sum(axis=1)


def _resid(params, x, t, w):
    return (_model(params, x) - t) * w


def _jac(params, x, t, w):
    Ka = len(params) // 3
    a, al, be = params[0::3][:Ka], params[1::3][:Ka], params[2::3][:Ka]
    arg = np.outer(x, al) + be[None, :]
    E = sp.erf(arg)
    G = (2.0 / np.sqrt(np.pi)) * np.exp(-np.minimum(arg * arg, 700.0))
    J = np.empty((len(x), 3 * Ka))
    J[:, 0::3] = E
    J[:, 1::3] = a[None, :] * G * x[:, None]
    J[:, 2::3] = a[None, :] * G
    return J * w[:, None]


def fit_cdf_atoms(cali, n_atoms=16, decimate=5):
    """Fit F_emp by a sum of erf atoms; returns (params, absmax_on_full_grid)."""
    cali = np.asarray(cali, dtype=np.float64)
    c = len(cali)
    srt = np.sort(cali)
    gaps = 0.5 * (srt[1:] + srt[:-1])
    xg_full = np.concatenate([srt, gaps])
    tg_full = np.concatenate([(np.arange(c) + 0.5) / c, (np.arange(c - 1) + 1.0) / c])
    order = np.argsort(xg_full)
    xg_full, tg_full = xg_full[order], tg_full[order]
    xg, tg = xg_full[::decimate], tg_full[::decimate]

    mu, sig = cali.mean(), cali.std()
    params = [0.5, 1.0 / (sig * np.sqrt(2)), -mu / (sig * np.sqrt(2))]
    wt = np.ones(len(xg))
    best = None
    while True:
        Ka = len(params) // 3
        res = least_squares(_resid, params, jac=_jac, args=(xg, tg, wt),
                            method="lm", max_nfev=25)
        params = list(res.x)
        r = _model(np.array(params), xg) - tg
        amax = np.abs(r).max()
        if best is None or amax < best[1]:
            best = (list(params), amax)
        if Ka >= n_atoms:
            break
        ipk = int(np.argmax(np.abs(r)))
        sgn = np.sign(r[ipk])
        lo = ipk
        while lo > 0 and r[lo - 1] * sgn > amax * 0.3:
            lo -= 1
        hi = ipk
        while hi < len(r) - 1 and r[hi + 1] * sgn > amax * 0.3:
            hi += 1
        width = max(xg[hi] - xg[lo], 1e-4)
        cpk = xg[ipk]
        params += [sgn * amax * 0.7, 1.0 / width, -cpk / width]
    params = np.array(best[0])
    rf = _model(params, xg_full) - tg_full
    return params, float(np.abs(rf).max())


# ----------------------------------------------------------------------------
# Bass kernel build
# ----------------------------------------------------------------------------
def _build_kernel(coefs, alphas, betas):
    """v9. Per group of 128 batch rows:
      - one 1 MB HWDGE load of the group's fp8 planes, laid out so SBUF
        partition p of tile t holds DRAM row (g*1024 + t*128 + p) of the
        [8192, 1000] (row, k)-collapsed matrix;
      - TensorE contracts each tile with a [128, 16] selection matrix
        (sel[p, c] = 1 iff p//8 == c) into psum[t*16:(t+1)*16, :] -- the
        K-sum lands in PSUM as exact f32;
      - ScalarE computes one erf atom per pass reading PSUM;
      - DVE combines atoms into p = 0.5 - sum_j a_j e_j, fp16 out.
    """
    import concourse.bacc as bacc
    import concourse.tile as tile
    import concourse.bass as bass
    from concourse import mybir

    n_atoms = len(coefs)
    NU = KK // 2          # 4 DoubleRow matmuls per chunk (2 k-rows per PE cell)
    LP = 1008             # padded row slot: 16-aligned Ko step, host pre-pads
    nc = bacc.Bacc("TRN2", target_bir_lowering=False, debug=False,
                   num_devices=N_CORES)
    # x is host-swizzled AND host-padded: row g*128 + p holds the 8 k-planes of
    # the group's DoubleRow pairs in 1008B slots -- col (u*2+j)*LP + l =
    # X2[g*1024 + u*256 + 2p + j, l] -- so every load is fully contiguous on
    # both the DRAM and SBUF side (one dense 8064B line per partition).
    x_in = nc.dram_tensor("x", [ROWS_PER_CORE, KK * LP], mybir.dt.float8e4,
                          kind="ExternalInput").ap()
    sel_in = nc.dram_tensor("sel", [128, NU, 2, 128], mybir.dt.float8e4,
                            kind="ExternalInput").ap()
    biases_in = nc.dram_tensor("biases", [n_atoms], mybir.dt.float32,
                               kind="ExternalInput").ap()
    # output is uint8: o = p*250 + 2.5, dequantized on host. The range
    # [2.25, 252.8] keeps clear of 0/255 so wrap-vs-saturate never matters.
    p_out = nc.dram_tensor("p", [ROWS_PER_CORE, L], mybir.dt.uint8,
                           kind="ExternalOutput").ap()

    with tile.TileContext(nc) as tc:
        with (
            tc.tile_pool(name="singles", bufs=1) as singles,
            tc.tile_pool(name="stage", bufs=N_GROUPS) as stage_p,
            tc.tile_pool(name="epool", bufs=3) as e_p,
            tc.tile_pool(name="opool", bufs=3) as o_p,
            tc.tile_pool(name="ppool", bufs=4, space="PSUM") as ppool,
        ):
            sel_t = singles.tile([128, NU, 2, 128], mybir.dt.float8e4)
            bias_t = singles.tile([128, n_atoms], mybir.dt.float32)
            # sel/bias land first on the Scalar HWDGE ring: tiny transfers the
            # very first matmul (and erf) depend on
            nc.scalar.dma_start(out=sel_t, in_=sel_in)
            nc.scalar.dma_start(
                out=bias_t,
                in_=bass.AP(tensor=biases_in.tensor, offset=biases_in.offset,
                            ap=[[0, 128], biases_in.ap[0]]))
            sts = []
            for g in range(N_GROUPS):
                # stage tile: st[p, u, j, 0:L] = X2[g*1024 + u*256 + 2p + j, :]
                # (pair j packed per partition for DoubleRow; slot padded to LP
                #  so the Ko step is 16-aligned)
                st = stage_p.tile([128, NU, 2, LP], mybir.dt.float8e4,
                                  tag="st", name="stageT")
                # loads stripe across BOTH HWDGE rings (Sync + Scalar) so two
                # DMA queues stream concurrently toward the HBM limit
                ring = nc.sync if g % 2 == 0 else nc.scalar
                if g == 0:
                    # split the first load so group 0's first matmuls can
                    # start after half the data has landed
                    for h in range(2):
                        ring.dma_start(
                            out=st[:, 2 * h:2 * h + 2, :, :],
                            in_=bass.AP(
                                tensor=x_in.tensor,
                                offset=x_in.offset + h * 4 * LP,
                                ap=[[KK * LP, 128], [1, 4 * LP]]))
                else:
                    ring.dma_start(
                        out=st,
                        in_=x_in[g * 128:(g + 1) * 128, :])
                sts.append(st)

            for g in range(N_GROUPS):
                st = sts[g]
                psum_t = ppool.tile([128, L], mybir.dt.float32, tag="ps",
                                    name="psumA")
                for c0, cw in ((0, 512), (512, L - 512)):
                    for u in range(NU):
                        nc.tensor.matmul(
                            psum_t[:, c0:c0 + cw],
                            lhsT=sel_t[:, u, :, :],
                            rhs=st[:, u, :, c0:c0 + cw],
                            start=(u == 0), stop=(u == NU - 1),
                            perf_mode=mybir.MatmulPerfMode.DoubleRow)

                o_t = o_p.tile([128, L], mybir.dt.uint8, tag="ot",
                               name="outT")
                if n_atoms == 1:
                    e_t = e_p.tile([128, L], mybir.dt.float16, tag="e0",
                                   name="erf0")
                    nc.scalar.activation(
                        out=e_t, in_=psum_t,
                        func=mybir.ActivationFunctionType.Erf,
                        scale=float(alphas[0]), bias=bias_t[:, 0:1])
                    nc.vector.tensor_scalar(
                        out=o_t, in0=e_t, scalar1=float(coefs[0]) * 250.0,
                        scalar2=127.5,
                        op0=mybir.AluOpType.mult, op1=mybir.AluOpType.add)
                else:
                    acc = o_p.tile([128, L], mybir.dt.float16, tag="acc",
                                   name="accT")
                    for j in range(n_atoms):
                        e_t = e_p.tile([128, L], mybir.dt.float16,
                                       tag=f"e{j}", name=f"erf{j}")
                        nc.scalar.activation(
                            out=e_t, in_=psum_t,
                            func=mybir.ActivationFunctionType.Erf,
                            scale=float(alphas[j]), bias=bias_t[:, j:j + 1])
                        if j == 0:
                            nc.vector.tensor_scalar(
                                out=acc, in0=e_t,
                                scalar1=float(coefs[0]) * 250.0,
                                scalar2=127.5,
                                op0=mybir.AluOpType.mult,
                                op1=mybir.AluOpType.add)
                        else:
                            nc.vector.scalar_tensor_tensor(
                                out=(o_t if j == n_atoms - 1 else acc),
                                in0=e_t, scalar=float(coefs[j]) * 250.0,
                                in1=acc,
                                op0=mybir.AluOpType.mult,
                                op1=mybir.AluOpType.add)
                # all stores ride the Sync ring: the Scalar engine then runs
                # pure ACTIVATEs, so the ACT->DVE->store chain never stalls it
                nc.sync.dma_start(out=p_out[g * 128:(g + 1) * 128, :],
                                  in_=o_t)
    nc.compile()
    return nc


# ----------------------------------------------------------------------------
# Host-side prep shared by kernel() and test.py
# ----------------------------------------------------------------------------
def _prepare(x, cali):
    """Returns (nc, in_maps).  x: [B, KK, L] f32, cali: [C] f32."""
    import ml_dtypes

    params, absmax = fit_cdf_atoms(cali, n_atoms=2)
    if absmax > 4e-3:  # unlucky draw: spend more atoms
        params, absmax = fit_cdf_atoms(cali, n_atoms=6)
    a = params[0::3]
    alphas = params[1::3]
    betas = params[2::3]
    coefs = (-a).astype(np.float64)  # p = 1 - F = 0.5 - sum a_j erf(.)

    # fp8-e4m3 quantization with error feedback along k: the device sum of the
    # 8 planes equals the true sum minus only the final rounding residual.
    q = np.empty((B, KK, L), dtype=ml_dtypes.float8_e4m3)
    r = np.zeros((B, L), dtype=np.float32)
    for k in range(KK):
        v = x[:, k, :] + r
        qk = v.astype(ml_dtypes.float8_e4m3)
        q[:, k, :] = qk
        r = v - qk.astype(np.float32)

    # DoubleRow selection: matmul u covers DRAM rows u*256 + 2p + j of the
    # group; cell (p, pair j) contributes to out row c = u*32 + (2p+j)//8.
    sel = np.zeros((128, KK // 2, 2, 128), dtype=ml_dtypes.float8_e4m3)
    for p in range(128):
        for u in range(KK // 2):
            for j in range(2):
                sel[p, u, j, u * 32 + (2 * p + j) // 8] = 1.0

    nc = _build_kernel(coefs, alphas, betas)

    LP = 1008
    in_maps = []
    for i in range(N_CORES):
        shard = q[i * ROWS_PER_CORE:(i + 1) * ROWS_PER_CORE]
        # swizzle [g, u, p, j, l] -> [g, p, u, j, l] and pad each 1000B row to
        # a 1008B slot so the device load is contiguous on both sides
        xd = (shard.reshape(N_GROUPS, KK // 2, 128, 2, L)
              .transpose(0, 2, 1, 3, 4))
        xp = np.zeros((N_GROUPS, 128, KK // 2, 2, LP),
                      dtype=shard.dtype)
        xp[:, :, :, :, :L] = xd
        in_maps.append({
            "x": xp.reshape(ROWS_PER_CORE, KK * LP),
            "sel": sel,
            "biases": np.asarray(betas, dtype=np.float32),
        })
    return nc, in_maps


def kernel(**inputs) -> np.ndarray:
    from concourse.bass_utils import run_bass_kernel_spmd

    x = np.ascontiguousarray(np.asarray(inputs["nonconformity"], dtype=np.float32))
    cali = np.asarray(inputs["cali_nonconformity"], dtype=np.float32)
    assert x.shape == (B, KK, L), x.shape
    assert cali.shape == (C,), cali.shape

    nc, in_maps = _prepare(x, cali)
    res = run_bass_kernel_spmd(nc, in_maps, list(range(N_CORES)))
    out = np.concatenate(
        [np.asarray(res.results[i]["p"]) for i in range(N_CORES)], axis=0)
    return ((out.astype(np.float32) - 2.5) * (1.0 / 250.0)).astype(np.float32)


if __name__ == "__main__":
    rng = np.random.default_rng(1)
    x = rng.standard_normal((B, KK, L), dtype=np.float32)
    cali = rng.standard_normal(C, dtype=np.float32)
    p = kernel(nonconformity=x, label_sample=np.zeros(L, np.int32),
               cali_nonconformity=cali)
    tot = x.sum(axis=1, dtype=np.float32)
    ref = (C - np.searchsorted(np.sort(cali), tot, side="left")).astype(np.float32) / C
    err = np.abs(p - ref)
    print("abs max err:", err.max(), " mean:", err.mean())
    print("rel l2:", np.linalg.norm(p - ref) / np.linalg.norm(ref))
